# revision 26
# baseline (speedup 1.0000x reference)
"""MLA attention kernel for Trainium2 (8 NeuronCores, Bass/Tile).

Sharding: 8 cores = 2 batches x 4 kv-head-groups. Core i handles batch
i//4 and kv head g=i%4 (query heads 4g..4g+3). No collectives: the
row-parallel o_proj partials are summed on the host during the gather.

Host-side algebraic preprocessing (exact, no approximation):
  * The reference's apply_rope slices the rope cache with x.shape[-2],
    which is the HEAD axis - so each head uses the rope angle of
    position h, independent of sequence position. RoPE is therefore a
    constant per-head 2x2 block-rotation of output channels and is
    folded into Wqrope / Wkrope columns.
  * v is zero-padded to 192 dims before out@Wo, so only the first 128
    channels of each head's 192 output dims are nonzero: Wo shrinks to
    the 128-rows-per-head submatrix.
  * Wq/Wqrope (and Wkdec/Wkrope) merge into single [nope|rope]
    per-head column blocks; Wq and Wkvc further merge into one
    [q-cols | c-cols] projection so q and c come out of one pass.

On-device per core, all phases in one TileContext with no DRAM
round-trips (qT/cT/kT/v live in SBUF):
  A: stream hsT in 512-seq chunks; qT (per-head 128 nope + 64 rope
     tiles) and cT to SBUF. bf16 operands, fp32 PSUM.
  B: kT = Wk.T @ cT; v = cT.T @ Wv (keys on partitions).
  C: causal attention in scoresT layout (keys on partitions, queries
     free). Diagonal 128x512 blocks compute only the live query range;
     a single [128,128] triangle mask handles the transition block.
     Softmax denominator: DVE accumulates exp tiles, then ONE all-ones
     [128,128] matmul broadcasts the partition-sum to all partitions
     (J @ acc), avoiding per-block ones-matmuls and single-partition
     reciprocals. No max-subtraction: scores are bounded (~|2|).
  D: o_proj partial per 512-query block, overlapped with C.
"""

import os
import sys

import numpy as np

sys.path.insert(0, "/opt/trn_rl_repo")

P = 128
B, S, HID = 2, 2048, 2048
H, KV, HD, RD = 16, 4, 128, 64
DF = HD + RD  # 192
CD = 512
NH = H // KV  # heads per core = 4
NK = HID // P  # 16
NS = S // P  # 16
QB = 512
NQ = S // QB  # 4
QC = NH * DF  # 768 q cols per core
WC = QC + CD  # 1280 combined projection cols
SCALE = 1.0 / float(np.sqrt(DF))

_NC_CACHE = {}


def build_mla_nc(debug=False):
    import concourse.tile as tile
    from concourse import bacc
    import concourse.mybir as mybir

    F32 = mybir.dt.float32
    F32R = mybir.dt.float32r
    BF16 = mybir.dt.bfloat16
    AF = mybir.ActivationFunctionType

    nc = bacc.Bacc("TRN2", target_bir_lowering=False, debug=debug)

    hsT = nc.dram_tensor("hsT", [HID, S], BF16, kind="ExternalInput")
    Wqc = nc.dram_tensor("Wqc", [HID, WC], BF16, kind="ExternalInput")
    Wk = nc.dram_tensor("Wk_s", [CD, DF], BF16, kind="ExternalInput")
    Wv = nc.dram_tensor("Wv_s", [CD, HD], BF16, kind="ExternalInput")
    Wo = nc.dram_tensor("Wo_s", [NH * HD, HID], BF16, kind="ExternalInput")
    tri = nc.dram_tensor("tri", [P, P], BF16, kind="ExternalInput")
    out = nc.dram_tensor("out", [S, HID], BF16, kind="ExternalOutput")

    def mm(ps, lhsT, rhs, start, stop):
        nc.tensor.matmul(ps, lhsT, rhs, start=start, stop=stop)

    with tile.TileContext(nc) as tc:
        with tc.tile_pool(name="cons", bufs=1) as cons, \
             tc.tile_pool(name="qtp", bufs=1) as qtp, \
             tc.tile_pool(name="ctp", bufs=1) as ctp, \
             tc.tile_pool(name="ktp", bufs=1) as ktp, \
             tc.tile_pool(name="vp", bufs=1) as vp, \
             tc.tile_pool(name="outn", bufs=1) as outn, \
             tc.tile_pool(name="wop", bufs=1) as wop, \
             tc.tile_pool(name="wkvp", bufs=1) as wkvp:
            tri_sb = cons.tile([P, P], BF16)
            nc.sync.dma_start(out=tri_sb[:], in_=tri[:, :])
            ones_sq = cons.tile([P, P], BF16)
            nc.vector.memset(ones_sq[:], 1.0)

            qta_sb = qtp.tile([P, NH, S], BF16)
            qtb_sb = qtp.tile([P, NH // 2, S], BF16)
            ct_sb = ctp.tile([P, CD // P, S], BF16)
            kt_a = ktp.tile([P, S], BF16)
            kt_b = ktp.tile([P, S], BF16)
            v_sb = vp.tile([P, NS, HD], BF16)
            out_nT = outn.tile([P, NH, S], BF16)
            wo_sb = wop.tile([P, NH, HID], BF16)

            # ---- Phase A: qT / cT projections (streamed, SBUF-resident) ----
            with tc.tile_pool(name="wqcp", bufs=1) as wqcp, \
                 tc.tile_pool(name="hsp", bufs=3) as hsp, \
                 tc.tile_pool(name="ppA", bufs=2, space="PSUM") as ppA:
                wqc_sb = wqcp.tile([P, NK, WC], BF16)
                wk_sb = wkvp.tile([P, CD // P, DF], BF16)
                wv_sb = wkvp.tile([P, CD // P, HD], BF16)
                hs_tiles = []
                # DMA issue order: hs chunk 0 and the c-projection weight
                # columns first so the first matmul starts early; the rest
                # of the weights stream behind.
                hs_t = hsp.tile([P, NK, QB], BF16)
                nc.sync.dma_start(
                    out=hs_t[:],
                    in_=hsT[:, 0:QB].rearrange("(k p) s -> p k s", p=P),
                )
                nc.sync.dma_start(
                    out=wqc_sb[:, :, QC:QC + P],
                    in_=Wqc[:, QC:QC + P].rearrange("(k p) m -> p k m", p=P),
                )
                nc.sync.dma_start(
                    out=wqc_sb[:, :, QC + P:WC],
                    in_=Wqc[:, QC + P:WC].rearrange("(k p) m -> p k m", p=P),
                )
                hs_tiles.append(hs_t)
                nc.sync.dma_start(
                    out=wk_sb[:],
                    in_=Wk[:, :].rearrange("(c p) m -> p c m", p=P),
                )
                nc.sync.dma_start(
                    out=wv_sb[:],
                    in_=Wv[:, :].rearrange("(c p) m -> p c m", p=P),
                )
                nc.sync.dma_start(
                    out=wqc_sb[:, :, 0:QC],
                    in_=Wqc[:, 0:QC].rearrange("(k p) m -> p k m", p=P),
                )
                for n in range(NQ):
                    nsl = slice(n * QB, (n + 1) * QB)
                    if n > 0:
                        hs_t = hsp.tile([P, NK, QB], BF16)
                        nc.sync.dma_start(
                            out=hs_t[:],
                            in_=hsT[:, nsl].rearrange("(k p) s -> p k s", p=P),
                        )
                        hs_tiles.append(hs_t)
                    hs_t = hs_tiles[n]
                    for m in range(CD // P):
                        ps = ppA.tile([P, QB], F32, tag="psA")
                        for k in range(NK):
                            mm(
                                ps[:],
                                wqc_sb[:, k, QC + m * P:QC + (m + 1) * P],
                                hs_t[:, k, :],
                                k == 0,
                                k == NK - 1,
                            )
                        nc.scalar.activation(ct_sb[:, m, nsl], ps[:], AF.Copy)
                    for h in range(NH):
                        ps = ppA.tile([P, QB], F32, tag="psA")
                        for k in range(NK):
                            mm(
                                ps[:],
                                wqc_sb[:, k, P * h:P * (h + 1)],
                                hs_t[:, k, :],
                                k == 0,
                                k == NK - 1,
                            )
                        nc.scalar.activation(qta_sb[:, h, nsl], ps[:], AF.Copy)
                    for j in range(NH // 2):
                        ps = ppA.tile([P, QB], F32, tag="psA")
                        for k in range(NK):
                            mm(
                                ps[:],
                                wqc_sb[:, k, NH * P + P * j:NH * P + P * (j + 1)],
                                hs_t[:, k, :],
                                k == 0,
                                k == NK - 1,
                            )
                        nc.scalar.activation(qtb_sb[:, j, nsl], ps[:], AF.Copy)

            # o_proj weights arrive while attention runs
            nc.sync.dma_start(
                out=wo_sb[:],
                in_=Wo[:, :].rearrange("(h p) n -> p h n", p=P),
            )

            # ---- Phase B: kT = Wk.T @ cT ; v = cT.T @ Wv ----
            with tc.tile_pool(name="ppK", bufs=2, space="PSUM") as ppK, \
                 tc.tile_pool(name="ppV", bufs=2, space="PSUM") as ppV:
                for n in range(NQ):
                    nsl = slice(n * QB, (n + 1) * QB)
                    for mt, m0, msz in [(0, 0, P), (1, P, RD)]:
                        psk = ppK.tile([P, QB], F32)
                        for c in range(CD // P):
                            mm(
                                psk[:msz, :],
                                wk_sb[:, c, m0:m0 + msz],
                                ct_sb[:, c, nsl],
                                c == 0,
                                c == CD // P - 1,
                            )
                        dst = kt_a if mt == 0 else kt_b
                        nc.scalar.activation(dst[:msz, nsl], psk[:msz, :], AF.Copy)
                # duplicate the 64 rope rows onto partitions 64..127 so odd
                # heads' scores matmuls get matching operand partition bases
                nc.sync.dma_start(out=kt_b[RD:P, :], in_=kt_b[0:RD, :])
                for n in range(NQ):
                    nsl = slice(n * QB, (n + 1) * QB)
                    for kp in range(QB // P):
                        kpg = n * (QB // P) + kp
                        psv = ppV.tile([P, HD], F32)
                        for c in range(CD // P):
                            mm(
                                psv[:],
                                ct_sb[:, c, kpg * P:(kpg + 1) * P],
                                wv_sb[:, c, :],
                                c == 0,
                                c == CD // P - 1,
                            )
                        nc.scalar.activation(v_sb[:, kpg, :], psv[:], AF.Copy)

            # ---- Phase C: causal attention + Phase D: o_proj per qb ----
            with tc.tile_pool(name="expp", bufs=4) as expp, \
                 tc.tile_pool(name="accp", bufs=2) as accp, \
                 tc.tile_pool(name="rcbp", bufs=2) as rcbp, \
                 tc.tile_pool(name="stD", bufs=3) as stDp, \
                 tc.tile_pool(name="scps", bufs=2, space="PSUM") as scps, \
                 tc.tile_pool(name="pvps", bufs=3, space="PSUM") as pvps, \
                 tc.tile_pool(name="bcps", bufs=1, space="PSUM") as bcps, \
                 tc.tile_pool(name="ppD", bufs=2, space="PSUM") as ppD:
                def emit_D(qb):
                    for sb in range(QB // P):
                        sbg = qb * (QB // P) + sb
                        for nb in range(NQ):
                            ps = ppD.tile([P, QB], F32)
                            for h in range(NH):
                                mm(
                                    ps[:],
                                    out_nT[:, h, sbg * P:(sbg + 1) * P],
                                    wo_sb[:, h, nb * QB:(nb + 1) * QB],
                                    h == 0,
                                    h == NH - 1,
                                )
                            st = stDp.tile([P, QB], BF16)
                            # alternate engines: keeps the o_proj drain off
                            # the DVE critical path (accumulate adds + norm)
                            if nb % 2 == 0:
                                nc.scalar.activation(st[:], ps[:], AF.Copy)
                            else:
                                nc.vector.tensor_copy(st[:], ps[:])
                            nc.sync.dma_start(
                                out=out[
                                    sbg * P:(sbg + 1) * P, nb * QB:(nb + 1) * QB
                                ],
                                in_=st[:],
                            )

                for qb in range(NQ):
                    nkb = (QB // P) * (qb + 1)
                    for h in range(NH):
                        if h == 2 and qb > 0:
                            emit_D(qb - 1)
                        pv = pvps.tile([P, QB], F32)
                        acc = accp.tile([P, QB], BF16)
                        for kb in range(nkb):
                            t = kb - (QB // P) * qb
                            q0 = P * t if t >= 0 else 0
                            qsl = slice(q0, QB)
                            gsl = slice(qb * QB + q0, (qb + 1) * QB)
                            sc = scps.tile([P, QB], F32)
                            mm(
                                sc[:, qsl],
                                kt_a[:, kb * P:(kb + 1) * P],
                                qta_sb[:, h, gsl],
                                True,
                                False,
                            )
                            h2 = RD * (h % 2)
                            mm(
                                sc[:, qsl],
                                kt_b[h2:h2 + RD, kb * P:(kb + 1) * P],
                                qtb_sb[h2:h2 + RD, h // 2, gsl],
                                False,
                                True,
                            )
                            ex = expp.tile([P, QB], BF16)
                            nc.scalar.activation(
                                ex[:, qsl], sc[:, qsl], AF.Exp, scale=SCALE
                            )
                            if t >= 0:
                                nc.vector.tensor_mul(
                                    ex[:, q0:q0 + P], ex[:, q0:q0 + P], tri_sb[:]
                                )
                            mm(
                                pv[:, qsl],
                                v_sb[:, kb, :],
                                ex[:, qsl],
                                kb == 0,
                                kb == nkb - 1,
                            )
                            if kb == 0:
                                nc.vector.tensor_copy(acc[:], ex[:])
                            else:
                                nc.vector.tensor_add(
                                    acc[:, qsl], acc[:, qsl], ex[:, qsl]
                                )
                        bc = bcps.tile([P, QB], F32)
                        mm(bc[:], ones_sq[:], acc[:], True, True)
                        rcb = rcbp.tile([P, QB], F32)
                        nc.vector.reciprocal_approx_fast(rcb[:], bc[:])
                        nc.vector.tensor_mul(
                            out_nT[:, h, qb * QB:(qb + 1) * QB], pv[:], rcb[:]
                        )
                emit_D(NQ - 1)

    nc.compile()
    return nc


def get_nc(debug=False):
    key = bool(debug)
    if key not in _NC_CACHE:
        _NC_CACHE[key] = build_mla_nc(debug=debug)
    return _NC_CACHE[key]


def _rope_fold(W, n_heads, in_dim):
    """Fold the reference's (head-indexed) RoPE into projection columns.

    W: (in_dim, n_heads*RD). Returns W' with
    W'[:, h, 2i]   = W[:, h, 2i]*cos[h,i] - W[:, h, 2i+1]*sin[h,i]
    W'[:, h, 2i+1] = W[:, h, 2i]*sin[h,i] + W[:, h, 2i+1]*cos[h,i]
    where cos/sin use position index h (the reference bug).
    """
    freqs = 1.0 / (10000.0 ** (np.arange(0, RD, 2, dtype=np.float64) / RD))
    t = np.arange(n_heads, dtype=np.float64)
    f = np.outer(t, freqs)  # (n_heads, RD//2)
    cos = np.cos(f)
    sin = np.sin(f)
    W4 = W.astype(np.float64).reshape(in_dim, n_heads, RD // 2, 2)
    e, o = W4[..., 0], W4[..., 1]
    e2 = e * cos[None] - o * sin[None]
    o2 = e * sin[None] + o * cos[None]
    out = np.stack([e2, o2], axis=-1).reshape(in_dim, n_heads, RD)
    return out.astype(np.float32)


def _build_in_maps(hidden_states, Wq, Wkvc, Wkdec, Wvdec, Wqrope, Wkrope, Wo):
    import ml_dtypes

    bf16 = ml_dtypes.bfloat16

    Wqr = _rope_fold(np.asarray(Wqrope, np.float32), H, HID)  # (HID,H,RD)
    Wkr = _rope_fold(np.asarray(Wkrope, np.float32), KV, CD)  # (CD,KV,RD)
    Wq_full = np.concatenate(
        [np.asarray(Wq, np.float32).reshape(HID, H, HD), Wqr], axis=-1
    )  # (HID, H, DF)
    Wk_full = np.concatenate(
        [np.asarray(Wkdec, np.float32).reshape(CD, KV, HD), Wkr], axis=-1
    )  # (CD, KV, DF)
    Wv4 = np.asarray(Wvdec, np.float32).reshape(CD, KV, HD)
    Wo_eff = np.asarray(Wo, np.float32).reshape(H, DF, HID)[:, :HD, :]  # (H,HD,HID)
    Wkvc32 = np.asarray(Wkvc, np.float32)

    tri = np.tril(np.ones((P, P), np.float32)).T.astype(bf16)  # tri[p,j]=p<=j

    hs = np.asarray(hidden_states, np.float32)
    hsT_b = [np.ascontiguousarray(hs[b].T).astype(bf16) for b in range(B)]

    per_g = []
    for g in range(KV):
        Wg = Wq_full[:, NH * g:NH * (g + 1), :]  # (HID, NH, DF)
        # col order: 4 per-head nope blocks (128 each), then 2 rope-pair
        # blocks ([h0|h1] and [h2|h3], 128 each)
        qcols = np.concatenate(
            [Wg[:, h, :HD] for h in range(NH)]
            + [
                np.concatenate([Wg[:, 2 * j, HD:], Wg[:, 2 * j + 1, HD:]], axis=1)
                for j in range(NH // 2)
            ],
            axis=1,
        )  # (HID, QC)
        per_g.append(
            {
                "Wqc": np.ascontiguousarray(
                    np.concatenate([qcols, Wkvc32], axis=1)
                ).astype(bf16),
                "Wk_s": np.ascontiguousarray(Wk_full[:, g, :]).astype(bf16),
                "Wv_s": np.ascontiguousarray(Wv4[:, g, :]).astype(bf16),
                "Wo_s": np.ascontiguousarray(
                    Wo_eff[NH * g:NH * (g + 1)].reshape(NH * HD, HID)
                ).astype(bf16),
                "tri": tri,
            }
        )

    in_maps = []
    for core in range(8):
        b, g = core // 4, core % 4
        m = dict(per_g[g])
        m["hsT"] = hsT_b[b]
        in_maps.append(m)
    return in_maps


def _maybe_enable_ldw_opt():
    """Opt-in experiment: let walrus overlap LDWEIGHTS with matmuls."""
    if os.environ.get("MLA_LDWOPT") != "1":
        return
    from concourse import bass_utils as bu

    if getattr(bu, "_mla_ldw_patched", False):
        return
    orig = bu.run_command

    def patched(cmd, cwd=None):
        cmd = [
            "--enable-ldw-opt=true" if c == "--enable-ldw-opt=false" else c
            for c in cmd
        ]
        return orig(cmd, cwd=cwd)

    bu.run_command = patched
    bu._mla_ldw_patched = True


def kernel(hidden_states, Wq, Wkvc, Wkdec, Wvdec, Wqrope, Wkrope, Wo):
    from concourse.bass_utils import run_bass_kernel_spmd

    _maybe_enable_ldw_opt()

    in_maps = _build_in_maps(
        hidden_states, Wq, Wkvc, Wkdec, Wvdec, Wqrope, Wkrope, Wo
    )
    nc = get_nc(debug=False)
    trace = os.environ.get("MLA_TRACE") == "1"
    res = run_bass_kernel_spmd(nc, in_maps, list(range(8)), trace=trace)
    kernel._last_result = res
    out = np.empty((B, S, HID), np.float32)
    for b in range(B):
        acc = res.results[b * 4 + 0]["out"].astype(np.float32)
        for g in range(1, 4):
            acc = acc + res.results[b * 4 + g]["out"]
        out[b] = acc
    return out


# revision 28
# speedup vs baseline: 1.0056x; 1.0056x over previous
"""MLA attention kernel for Trainium2 (8 NeuronCores, Bass/Tile).

Sharding: 8 cores = 2 batches x 4 kv-head-groups. Core i handles batch
i//4 and kv head g=i%4 (query heads 4g..4g+3). No collectives: the
row-parallel o_proj partials are summed on the host during the gather.

Host-side algebraic preprocessing (exact, no approximation):
  * The reference's apply_rope slices the rope cache with x.shape[-2],
    which is the HEAD axis - so each head uses the rope angle of
    position h, independent of sequence position. RoPE is therefore a
    constant per-head 2x2 block-rotation of output channels and is
    folded into Wqrope / Wkrope columns.
  * v is zero-padded to 192 dims before out@Wo, so only the first 128
    channels of each head's 192 output dims are nonzero: Wo shrinks to
    the 128-rows-per-head submatrix.
  * Wq/Wqrope (and Wkdec/Wkrope) merge into single [nope|rope]
    per-head column blocks; Wq and Wkvc further merge into one
    [q-cols | c-cols] projection so q and c come out of one pass.

On-device per core, all phases in one TileContext with no DRAM
round-trips (qT/cT/kT/v live in SBUF):
  A: stream hsT in 512-seq chunks; qT (per-head 128 nope + 64 rope
     tiles) and cT to SBUF. bf16 operands, fp32 PSUM.
  B: kT = Wk.T @ cT; v = cT.T @ Wv (keys on partitions).
  C: causal attention in scoresT layout (keys on partitions, queries
     free). Diagonal 128x512 blocks compute only the live query range;
     a single [128,128] triangle mask handles the transition block.
     Softmax denominator: DVE accumulates exp tiles, then ONE all-ones
     [128,128] matmul broadcasts the partition-sum to all partitions
     (J @ acc), avoiding per-block ones-matmuls and single-partition
     reciprocals. No max-subtraction: scores are bounded (~|2|).
  D: o_proj partial per 512-query block, overlapped with C.
"""

import os
import sys

import numpy as np

sys.path.insert(0, "/opt/trn_rl_repo")

P = 128
B, S, HID = 2, 2048, 2048
H, KV, HD, RD = 16, 4, 128, 64
DF = HD + RD  # 192
CD = 512
NH = H // KV  # heads per core = 4
NK = HID // P  # 16
NS = S // P  # 16
QB = 512
NQ = S // QB  # 4
QC = NH * DF  # 768 q cols per core
WC = QC + CD  # 1280 combined projection cols
SCALE = 1.0 / float(np.sqrt(DF))

_NC_CACHE = {}


def build_mla_nc(debug=False):
    import concourse.tile as tile
    from concourse import bacc
    import concourse.mybir as mybir

    F32 = mybir.dt.float32
    F32R = mybir.dt.float32r
    BF16 = mybir.dt.bfloat16
    AF = mybir.ActivationFunctionType

    nc = bacc.Bacc("TRN2", target_bir_lowering=False, debug=debug)

    hsT = nc.dram_tensor("hsT", [HID, S], BF16, kind="ExternalInput")
    Wqc = nc.dram_tensor("Wqc", [HID, WC], BF16, kind="ExternalInput")
    Wk = nc.dram_tensor("Wk_s", [CD, DF], BF16, kind="ExternalInput")
    Wv = nc.dram_tensor("Wv_s", [CD, HD], BF16, kind="ExternalInput")
    Wo = nc.dram_tensor("Wo_s", [NH * HD, HID], BF16, kind="ExternalInput")
    tri = nc.dram_tensor("tri", [P, P], BF16, kind="ExternalInput")
    out = nc.dram_tensor("out", [S, HID], BF16, kind="ExternalOutput")

    def mm(ps, lhsT, rhs, start, stop):
        nc.tensor.matmul(ps, lhsT, rhs, start=start, stop=stop)

    with tile.TileContext(nc) as tc:
        with tc.tile_pool(name="cons", bufs=1) as cons, \
             tc.tile_pool(name="qtp", bufs=1) as qtp, \
             tc.tile_pool(name="ctp", bufs=1) as ctp, \
             tc.tile_pool(name="ktp", bufs=1) as ktp, \
             tc.tile_pool(name="vp", bufs=1) as vp, \
             tc.tile_pool(name="outn", bufs=1) as outn, \
             tc.tile_pool(name="wop", bufs=1) as wop, \
             tc.tile_pool(name="wkvp", bufs=1) as wkvp:
            tri_sb = cons.tile([P, P], BF16)
            nc.sync.dma_start(out=tri_sb[:], in_=tri[:, :])
            ones_sq = cons.tile([P, P], BF16)
            nc.vector.memset(ones_sq[:], 1.0)

            qta_sb = qtp.tile([P, NH, S], BF16)
            qtb_sb = qtp.tile([P, NH // 2, S], BF16)
            ct_sb = ctp.tile([P, CD // P, S], BF16)
            kt_a = ktp.tile([P, S], BF16)
            kt_b = ktp.tile([P, S], BF16)
            v_sb = vp.tile([P, NS, HD], BF16)
            out_nT = outn.tile([P, NH, S], BF16)
            wo_sb = wop.tile([P, NH, HID], BF16)

            # ---- Phase A: qT / cT projections (streamed, SBUF-resident) ----
            with tc.tile_pool(name="wqcp", bufs=1) as wqcp, \
                 tc.tile_pool(name="hsp", bufs=3) as hsp, \
                 tc.tile_pool(name="ppA", bufs=2, space="PSUM") as ppA:
                wqc_sb = wqcp.tile([P, NK, WC], BF16)
                wk_sb = wkvp.tile([P, CD // P, DF], BF16)
                wv_sb = wkvp.tile([P, CD // P, HD], BF16)
                hs_tiles = []
                # DMA issue order: hs chunk 0 and the c-projection weight
                # columns first so the first matmul starts early; the rest
                # of the weights stream behind.
                hs_t = hsp.tile([P, NK, QB], BF16)
                nc.sync.dma_start(
                    out=hs_t[:],
                    in_=hsT[:, 0:QB].rearrange("(k p) s -> p k s", p=P),
                )
                nc.sync.dma_start(
                    out=wqc_sb[:, :, QC:QC + P],
                    in_=Wqc[:, QC:QC + P].rearrange("(k p) m -> p k m", p=P),
                )
                nc.sync.dma_start(
                    out=wqc_sb[:, :, QC + P:WC],
                    in_=Wqc[:, QC + P:WC].rearrange("(k p) m -> p k m", p=P),
                )
                hs_tiles.append(hs_t)
                nc.sync.dma_start(
                    out=wk_sb[:],
                    in_=Wk[:, :].rearrange("(c p) m -> p c m", p=P),
                )
                nc.sync.dma_start(
                    out=wv_sb[:],
                    in_=Wv[:, :].rearrange("(c p) m -> p c m", p=P),
                )
                nc.sync.dma_start(
                    out=wqc_sb[:, :, 0:QC],
                    in_=Wqc[:, 0:QC].rearrange("(k p) m -> p k m", p=P),
                )
                for n in range(NQ):
                    nsl = slice(n * QB, (n + 1) * QB)
                    if n > 0:
                        hs_t = hsp.tile([P, NK, QB], BF16)
                        nc.sync.dma_start(
                            out=hs_t[:],
                            in_=hsT[:, nsl].rearrange("(k p) s -> p k s", p=P),
                        )
                        hs_tiles.append(hs_t)
                    hs_t = hs_tiles[n]
                    for m in range(CD // P):
                        ps = ppA.tile([P, QB], F32, tag="psA")
                        for k in range(NK):
                            mm(
                                ps[:],
                                wqc_sb[:, k, QC + m * P:QC + (m + 1) * P],
                                hs_t[:, k, :],
                                k == 0,
                                k == NK - 1,
                            )
                        nc.scalar.activation(ct_sb[:, m, nsl], ps[:], AF.Copy)
                    for h in range(NH):
                        ps = ppA.tile([P, QB], F32, tag="psA")
                        for k in range(NK):
                            mm(
                                ps[:],
                                wqc_sb[:, k, P * h:P * (h + 1)],
                                hs_t[:, k, :],
                                k == 0,
                                k == NK - 1,
                            )
                        nc.scalar.activation(qta_sb[:, h, nsl], ps[:], AF.Copy)
                    for j in range(NH // 2):
                        ps = ppA.tile([P, QB], F32, tag="psA")
                        for k in range(NK):
                            mm(
                                ps[:],
                                wqc_sb[:, k, NH * P + P * j:NH * P + P * (j + 1)],
                                hs_t[:, k, :],
                                k == 0,
                                k == NK - 1,
                            )
                        nc.scalar.activation(qtb_sb[:, j, nsl], ps[:], AF.Copy)

            # o_proj weights arrive while attention runs
            nc.sync.dma_start(
                out=wo_sb[:],
                in_=Wo[:, :].rearrange("(h p) n -> p h n", p=P),
            )

            # ---- Phase B: kT = Wk.T @ cT ; v = cT.T @ Wv ----
            with tc.tile_pool(name="ppK", bufs=3, space="PSUM") as ppK, \
                 tc.tile_pool(name="ppV", bufs=3, space="PSUM") as ppV:
                for n in range(NQ):
                    nsl = slice(n * QB, (n + 1) * QB)
                    for mt, m0, msz in [(0, 0, P), (1, P, RD)]:
                        psk = ppK.tile([P, QB], F32)
                        for c in range(CD // P):
                            mm(
                                psk[:msz, :],
                                wk_sb[:, c, m0:m0 + msz],
                                ct_sb[:, c, nsl],
                                c == 0,
                                c == CD // P - 1,
                            )
                        dst = kt_a if mt == 0 else kt_b
                        nc.scalar.activation(dst[:msz, nsl], psk[:msz, :], AF.Copy)
                # duplicate the 64 rope rows onto partitions 64..127 so odd
                # heads' scores matmuls get matching operand partition bases
                nc.sync.dma_start(out=kt_b[RD:P, :], in_=kt_b[0:RD, :])
                for n in range(NQ):
                    nsl = slice(n * QB, (n + 1) * QB)
                    for kp in range(QB // P):
                        kpg = n * (QB // P) + kp
                        psv = ppV.tile([P, HD], F32)
                        for c in range(CD // P):
                            mm(
                                psv[:],
                                ct_sb[:, c, kpg * P:(kpg + 1) * P],
                                wv_sb[:, c, :],
                                c == 0,
                                c == CD // P - 1,
                            )
                        nc.scalar.activation(v_sb[:, kpg, :], psv[:], AF.Copy)

            # ---- Phase C: causal attention + Phase D: o_proj per qb ----
            with tc.tile_pool(name="expp", bufs=4) as expp, \
                 tc.tile_pool(name="accp", bufs=2) as accp, \
                 tc.tile_pool(name="rcbp", bufs=2) as rcbp, \
                 tc.tile_pool(name="stD", bufs=3) as stDp, \
                 tc.tile_pool(name="scps", bufs=2, space="PSUM") as scps, \
                 tc.tile_pool(name="pvps", bufs=3, space="PSUM") as pvps, \
                 tc.tile_pool(name="bcps", bufs=1, space="PSUM") as bcps, \
                 tc.tile_pool(name="ppD", bufs=2, space="PSUM") as ppD:
                def emit_D(qb, sbs=None):
                    for sb in sbs if sbs is not None else range(QB // P):
                        sbg = qb * (QB // P) + sb
                        for nb in range(NQ):
                            ps = ppD.tile([P, QB], F32)
                            for h in range(NH):
                                mm(
                                    ps[:],
                                    out_nT[:, h, sbg * P:(sbg + 1) * P],
                                    wo_sb[:, h, nb * QB:(nb + 1) * QB],
                                    h == 0,
                                    h == NH - 1,
                                )
                            st = stDp.tile([P, QB], BF16)
                            nc.vector.tensor_copy(st[:], ps[:])
                            nc.sync.dma_start(
                                out=out[
                                    sbg * P:(sbg + 1) * P, nb * QB:(nb + 1) * QB
                                ],
                                in_=st[:],
                            )

                for qb in range(NQ):
                    nkb = (QB // P) * (qb + 1)
                    for h in range(NH):
                        # two half-bursts: keeps the o_proj drain copies from
                        # piling onto the DVE queue all at once
                        if h == 1 and qb > 0:
                            emit_D(qb - 1, sbs=(0, 1))
                        if h == 3 and qb > 0:
                            emit_D(qb - 1, sbs=(2, 3))
                        pv = pvps.tile([P, QB], F32)
                        acc = accp.tile([P, QB], BF16)
                        for kb in range(nkb):
                            t = kb - (QB // P) * qb
                            q0 = P * t if t >= 0 else 0
                            qsl = slice(q0, QB)
                            gsl = slice(qb * QB + q0, (qb + 1) * QB)
                            sc = scps.tile([P, QB], F32)
                            mm(
                                sc[:, qsl],
                                kt_a[:, kb * P:(kb + 1) * P],
                                qta_sb[:, h, gsl],
                                True,
                                False,
                            )
                            h2 = RD * (h % 2)
                            mm(
                                sc[:, qsl],
                                kt_b[h2:h2 + RD, kb * P:(kb + 1) * P],
                                qtb_sb[h2:h2 + RD, h // 2, gsl],
                                False,
                                True,
                            )
                            ex = expp.tile([P, QB], BF16)
                            nc.scalar.activation(
                                ex[:, qsl], sc[:, qsl], AF.Exp, scale=SCALE
                            )
                            if t >= 0:
                                nc.vector.tensor_mul(
                                    ex[:, q0:q0 + P], ex[:, q0:q0 + P], tri_sb[:]
                                )
                            mm(
                                pv[:, qsl],
                                v_sb[:, kb, :],
                                ex[:, qsl],
                                kb == 0,
                                kb == nkb - 1,
                            )
                            if kb == 0:
                                nc.vector.tensor_copy(acc[:], ex[:])
                            else:
                                nc.vector.tensor_add(
                                    acc[:, qsl], acc[:, qsl], ex[:, qsl]
                                )
                        bc = bcps.tile([P, QB], F32)
                        mm(bc[:], ones_sq[:], acc[:], True, True)
                        rcb = rcbp.tile([P, QB], F32)
                        nc.vector.reciprocal_approx_fast(rcb[:], bc[:])
                        nc.vector.tensor_mul(
                            out_nT[:, h, qb * QB:(qb + 1) * QB], pv[:], rcb[:]
                        )
                emit_D(NQ - 1)

    nc.compile()
    return nc


def get_nc(debug=False):
    key = bool(debug)
    if key not in _NC_CACHE:
        _NC_CACHE[key] = build_mla_nc(debug=debug)
    return _NC_CACHE[key]


def _rope_fold(W, n_heads, in_dim):
    """Fold the reference's (head-indexed) RoPE into projection columns.

    W: (in_dim, n_heads*RD). Returns W' with
    W'[:, h, 2i]   = W[:, h, 2i]*cos[h,i] - W[:, h, 2i+1]*sin[h,i]
    W'[:, h, 2i+1] = W[:, h, 2i]*sin[h,i] + W[:, h, 2i+1]*cos[h,i]
    where cos/sin use position index h (the reference bug).
    """
    freqs = 1.0 / (10000.0 ** (np.arange(0, RD, 2, dtype=np.float64) / RD))
    t = np.arange(n_heads, dtype=np.float64)
    f = np.outer(t, freqs)  # (n_heads, RD//2)
    cos = np.cos(f)
    sin = np.sin(f)
    W4 = W.astype(np.float64).reshape(in_dim, n_heads, RD // 2, 2)
    e, o = W4[..., 0], W4[..., 1]
    e2 = e * cos[None] - o * sin[None]
    o2 = e * sin[None] + o * cos[None]
    out = np.stack([e2, o2], axis=-1).reshape(in_dim, n_heads, RD)
    return out.astype(np.float32)


def _build_in_maps(hidden_states, Wq, Wkvc, Wkdec, Wvdec, Wqrope, Wkrope, Wo):
    import ml_dtypes

    bf16 = ml_dtypes.bfloat16

    Wqr = _rope_fold(np.asarray(Wqrope, np.float32), H, HID)  # (HID,H,RD)
    Wkr = _rope_fold(np.asarray(Wkrope, np.float32), KV, CD)  # (CD,KV,RD)
    Wq_full = np.concatenate(
        [np.asarray(Wq, np.float32).reshape(HID, H, HD), Wqr], axis=-1
    )  # (HID, H, DF)
    Wk_full = np.concatenate(
        [np.asarray(Wkdec, np.float32).reshape(CD, KV, HD), Wkr], axis=-1
    )  # (CD, KV, DF)
    Wv4 = np.asarray(Wvdec, np.float32).reshape(CD, KV, HD)
    Wo_eff = np.asarray(Wo, np.float32).reshape(H, DF, HID)[:, :HD, :]  # (H,HD,HID)
    Wkvc32 = np.asarray(Wkvc, np.float32)

    tri = np.tril(np.ones((P, P), np.float32)).T.astype(bf16)  # tri[p,j]=p<=j

    hs = np.asarray(hidden_states, np.float32)
    hsT_b = [np.ascontiguousarray(hs[b].T).astype(bf16) for b in range(B)]

    per_g = []
    for g in range(KV):
        Wg = Wq_full[:, NH * g:NH * (g + 1), :]  # (HID, NH, DF)
        # col order: 4 per-head nope blocks (128 each), then 2 rope-pair
        # blocks ([h0|h1] and [h2|h3], 128 each)
        qcols = np.concatenate(
            [Wg[:, h, :HD] for h in range(NH)]
            + [
                np.concatenate([Wg[:, 2 * j, HD:], Wg[:, 2 * j + 1, HD:]], axis=1)
                for j in range(NH // 2)
            ],
            axis=1,
        )  # (HID, QC)
        per_g.append(
            {
                "Wqc": np.ascontiguousarray(
                    np.concatenate([qcols, Wkvc32], axis=1)
                ).astype(bf16),
                "Wk_s": np.ascontiguousarray(Wk_full[:, g, :]).astype(bf16),
                "Wv_s": np.ascontiguousarray(Wv4[:, g, :]).astype(bf16),
                "Wo_s": np.ascontiguousarray(
                    Wo_eff[NH * g:NH * (g + 1)].reshape(NH * HD, HID)
                ).astype(bf16),
                "tri": tri,
            }
        )

    in_maps = []
    for core in range(8):
        b, g = core // 4, core % 4
        m = dict(per_g[g])
        m["hsT"] = hsT_b[b]
        in_maps.append(m)
    return in_maps


def _maybe_enable_ldw_opt():
    """Opt-in experiment: let walrus overlap LDWEIGHTS with matmuls."""
    if os.environ.get("MLA_LDWOPT") != "1":
        return
    from concourse import bass_utils as bu

    if getattr(bu, "_mla_ldw_patched", False):
        return
    orig = bu.run_command

    def patched(cmd, cwd=None):
        cmd = [
            "--enable-ldw-opt=true" if c == "--enable-ldw-opt=false" else c
            for c in cmd
        ]
        return orig(cmd, cwd=cwd)

    bu.run_command = patched
    bu._mla_ldw_patched = True


def kernel(hidden_states, Wq, Wkvc, Wkdec, Wvdec, Wqrope, Wkrope, Wo):
    from concourse.bass_utils import run_bass_kernel_spmd

    _maybe_enable_ldw_opt()

    in_maps = _build_in_maps(
        hidden_states, Wq, Wkvc, Wkdec, Wvdec, Wqrope, Wkrope, Wo
    )
    nc = get_nc(debug=False)
    trace = os.environ.get("MLA_TRACE") == "1"
    res = run_bass_kernel_spmd(nc, in_maps, list(range(8)), trace=trace)
    kernel._last_result = res
    out = np.empty((B, S, HID), np.float32)
    for b in range(B):
        acc = res.results[b * 4 + 0]["out"].astype(np.float32)
        for g in range(1, 4):
            acc = acc + res.results[b * 4 + g]["out"]
        out[b] = acc
    return out


# revision 29
# speedup vs baseline: 1.0089x; 1.0032x over previous
"""MLA attention kernel for Trainium2 (8 NeuronCores, Bass/Tile).

Sharding: 8 cores = 2 batches x 4 kv-head-groups. Core i handles batch
i//4 and kv head g=i%4 (query heads 4g..4g+3). No collectives: the
row-parallel o_proj partials are summed on the host during the gather.

Host-side algebraic preprocessing (exact, no approximation):
  * The reference's apply_rope slices the rope cache with x.shape[-2],
    which is the HEAD axis - so each head uses the rope angle of
    position h, independent of sequence position. RoPE is therefore a
    constant per-head 2x2 block-rotation of output channels and is
    folded into Wqrope / Wkrope columns.
  * v is zero-padded to 192 dims before out@Wo, so only the first 128
    channels of each head's 192 output dims are nonzero: Wo shrinks to
    the 128-rows-per-head submatrix.
  * Wq/Wqrope (and Wkdec/Wkrope) merge into single [nope|rope]
    per-head column blocks; Wq and Wkvc further merge into one
    [q-cols | c-cols] projection so q and c come out of one pass.

On-device per core, all phases in one TileContext with no DRAM
round-trips (qT/cT/kT/v live in SBUF):
  A: stream hsT in 512-seq chunks; qT (per-head 128 nope + 64 rope
     tiles) and cT to SBUF. bf16 operands, fp32 PSUM.
  B: kT = Wk.T @ cT; v = cT.T @ Wv (keys on partitions).
  C: causal attention in scoresT layout (keys on partitions, queries
     free). Diagonal 128x512 blocks compute only the live query range;
     a single [128,128] triangle mask handles the transition block.
     Softmax denominator: DVE accumulates exp tiles, then ONE all-ones
     [128,128] matmul broadcasts the partition-sum to all partitions
     (J @ acc), avoiding per-block ones-matmuls and single-partition
     reciprocals. No max-subtraction: scores are bounded (~|2|).
  D: o_proj partial per 512-query block, overlapped with C.
"""

import os
import sys

import numpy as np

sys.path.insert(0, "/opt/trn_rl_repo")

P = 128
B, S, HID = 2, 2048, 2048
H, KV, HD, RD = 16, 4, 128, 64
DF = HD + RD  # 192
CD = 512
NH = H // KV  # heads per core = 4
NK = HID // P  # 16
NS = S // P  # 16
QB = 512
NQ = S // QB  # 4
QC = NH * DF  # 768 q cols per core
WC = QC + CD  # 1280 combined projection cols
SCALE = 1.0 / float(np.sqrt(DF))

_NC_CACHE = {}


def build_mla_nc(debug=False):
    import concourse.tile as tile
    from concourse import bacc
    import concourse.mybir as mybir

    F32 = mybir.dt.float32
    F32R = mybir.dt.float32r
    BF16 = mybir.dt.bfloat16
    AF = mybir.ActivationFunctionType

    nc = bacc.Bacc("TRN2", target_bir_lowering=False, debug=debug)

    hsT = nc.dram_tensor("hsT", [HID, S], BF16, kind="ExternalInput")
    Wqc = nc.dram_tensor("Wqc", [HID, WC], BF16, kind="ExternalInput")
    Wk = nc.dram_tensor("Wk_s", [CD, DF], BF16, kind="ExternalInput")
    Wv = nc.dram_tensor("Wv_s", [CD, HD], BF16, kind="ExternalInput")
    Wo = nc.dram_tensor("Wo_s", [NH * HD, HID], BF16, kind="ExternalInput")
    tri = nc.dram_tensor("tri", [P, P], BF16, kind="ExternalInput")
    out = nc.dram_tensor("out", [S, HID], BF16, kind="ExternalOutput")

    def mm(ps, lhsT, rhs, start, stop):
        nc.tensor.matmul(ps, lhsT, rhs, start=start, stop=stop)

    with tile.TileContext(nc) as tc:
        with tc.tile_pool(name="cons", bufs=1) as cons, \
             tc.tile_pool(name="qtp", bufs=1) as qtp, \
             tc.tile_pool(name="ctp", bufs=1) as ctp, \
             tc.tile_pool(name="ktp", bufs=1) as ktp, \
             tc.tile_pool(name="vp", bufs=1) as vp, \
             tc.tile_pool(name="outn", bufs=1) as outn, \
             tc.tile_pool(name="wop", bufs=1) as wop, \
             tc.tile_pool(name="wkvp", bufs=1) as wkvp:
            tri_sb = cons.tile([P, P], BF16)
            nc.sync.dma_start(out=tri_sb[:], in_=tri[:, :])
            ones_sq = cons.tile([P, P], BF16)
            nc.vector.memset(ones_sq[:], 1.0)

            qta_sb = qtp.tile([P, NH, S], BF16)
            qtb_sb = qtp.tile([P, NH // 2, S], BF16)
            ct_sb = ctp.tile([P, CD // P, S], BF16)
            kt_a = ktp.tile([P, S], BF16)
            kt_b = ktp.tile([P, S], BF16)
            v_sb = vp.tile([P, NS, HD], BF16)
            out_nT = outn.tile([P, NH, S], BF16)
            wo_sb = wop.tile([P, NH, HID], BF16)

            # ---- Phase A: qT / cT projections (streamed, SBUF-resident) ----
            with tc.tile_pool(name="wqcp", bufs=1) as wqcp, \
                 tc.tile_pool(name="hsp", bufs=3) as hsp, \
                 tc.tile_pool(name="ppA", bufs=2, space="PSUM") as ppA:
                wqc_sb = wqcp.tile([P, NK, WC], BF16)
                wk_sb = wkvp.tile([P, CD // P, DF], BF16)
                wv_sb = wkvp.tile([P, CD // P, HD], BF16)
                hs_tiles = []
                # DMA issue order: hs chunk 0 and the c-projection weight
                # columns first so the first matmul starts early; the rest
                # of the weights stream behind.
                hs_t = hsp.tile([P, NK, QB], BF16)
                nc.sync.dma_start(
                    out=hs_t[:],
                    in_=hsT[:, 0:QB].rearrange("(k p) s -> p k s", p=P),
                )
                nc.sync.dma_start(
                    out=wqc_sb[:, :, QC:QC + P],
                    in_=Wqc[:, QC:QC + P].rearrange("(k p) m -> p k m", p=P),
                )
                nc.sync.dma_start(
                    out=wqc_sb[:, :, QC + P:WC],
                    in_=Wqc[:, QC + P:WC].rearrange("(k p) m -> p k m", p=P),
                )
                hs_tiles.append(hs_t)
                nc.sync.dma_start(
                    out=wk_sb[:],
                    in_=Wk[:, :].rearrange("(c p) m -> p c m", p=P),
                )
                nc.sync.dma_start(
                    out=wv_sb[:],
                    in_=Wv[:, :].rearrange("(c p) m -> p c m", p=P),
                )
                nc.sync.dma_start(
                    out=wqc_sb[:, :, 0:QC],
                    in_=Wqc[:, 0:QC].rearrange("(k p) m -> p k m", p=P),
                )
                for n in range(NQ):
                    nsl = slice(n * QB, (n + 1) * QB)
                    if n > 0:
                        hs_t = hsp.tile([P, NK, QB], BF16)
                        nc.sync.dma_start(
                            out=hs_t[:],
                            in_=hsT[:, nsl].rearrange("(k p) s -> p k s", p=P),
                        )
                        hs_tiles.append(hs_t)
                    hs_t = hs_tiles[n]
                    for m in range(CD // P):
                        ps = ppA.tile([P, QB], F32, tag="psA")
                        for k in range(NK):
                            mm(
                                ps[:],
                                wqc_sb[:, k, QC + m * P:QC + (m + 1) * P],
                                hs_t[:, k, :],
                                k == 0,
                                k == NK - 1,
                            )
                        nc.scalar.activation(ct_sb[:, m, nsl], ps[:], AF.Copy)
                    for h in range(NH):
                        ps = ppA.tile([P, QB], F32, tag="psA")
                        for k in range(NK):
                            mm(
                                ps[:],
                                wqc_sb[:, k, P * h:P * (h + 1)],
                                hs_t[:, k, :],
                                k == 0,
                                k == NK - 1,
                            )
                        nc.scalar.activation(qta_sb[:, h, nsl], ps[:], AF.Copy)
                    for j in range(NH // 2):
                        ps = ppA.tile([P, QB], F32, tag="psA")
                        for k in range(NK):
                            mm(
                                ps[:],
                                wqc_sb[:, k, NH * P + P * j:NH * P + P * (j + 1)],
                                hs_t[:, k, :],
                                k == 0,
                                k == NK - 1,
                            )
                        nc.scalar.activation(qtb_sb[:, j, nsl], ps[:], AF.Copy)

            # o_proj weights arrive while attention runs
            nc.sync.dma_start(
                out=wo_sb[:],
                in_=Wo[:, :].rearrange("(h p) n -> p h n", p=P),
            )

            # ---- Phase B: kT = Wk.T @ cT ; v = cT.T @ Wv ----
            with tc.tile_pool(name="ppK", bufs=3, space="PSUM") as ppK, \
                 tc.tile_pool(name="ppV", bufs=3, space="PSUM") as ppV:
                for n in range(NQ):
                    nsl = slice(n * QB, (n + 1) * QB)
                    for mt, m0, msz in [(0, 0, P), (1, P, RD)]:
                        psk = ppK.tile([P, QB], F32)
                        for c in range(CD // P):
                            mm(
                                psk[:msz, :],
                                wk_sb[:, c, m0:m0 + msz],
                                ct_sb[:, c, nsl],
                                c == 0,
                                c == CD // P - 1,
                            )
                        dst = kt_a if mt == 0 else kt_b
                        nc.scalar.activation(dst[:msz, nsl], psk[:msz, :], AF.Copy)
                # duplicate the 64 rope rows onto partitions 64..127 so odd
                # heads' scores matmuls get matching operand partition bases
                nc.sync.dma_start(out=kt_b[RD:P, :], in_=kt_b[0:RD, :])
                for n in range(NQ):
                    nsl = slice(n * QB, (n + 1) * QB)
                    for kp in range(QB // P):
                        kpg = n * (QB // P) + kp
                        psv = ppV.tile([P, HD], F32)
                        for c in range(CD // P):
                            mm(
                                psv[:],
                                ct_sb[:, c, kpg * P:(kpg + 1) * P],
                                wv_sb[:, c, :],
                                c == 0,
                                c == CD // P - 1,
                            )
                        nc.scalar.activation(v_sb[:, kpg, :], psv[:], AF.Copy)

            # ---- Phase C: causal attention + Phase D: o_proj per qb ----
            with tc.tile_pool(name="expp", bufs=4) as expp, \
                 tc.tile_pool(name="accp", bufs=2) as accp, \
                 tc.tile_pool(name="rcbp", bufs=2) as rcbp, \
                 tc.tile_pool(name="stD", bufs=3) as stDp, \
                 tc.tile_pool(name="scps", bufs=2, space="PSUM") as scps, \
                 tc.tile_pool(name="pvps", bufs=3, space="PSUM") as pvps, \
                 tc.tile_pool(name="bcps", bufs=1, space="PSUM") as bcps, \
                 tc.tile_pool(name="ppD", bufs=2, space="PSUM") as ppD:
                def emit_D(qb, sbs=None):
                    for sb in sbs if sbs is not None else range(QB // P):
                        sbg = qb * (QB // P) + sb
                        for nb in range(NQ):
                            ps = ppD.tile([P, QB], F32)
                            for h in range(NH):
                                mm(
                                    ps[:],
                                    out_nT[:, h, sbg * P:(sbg + 1) * P],
                                    wo_sb[:, h, nb * QB:(nb + 1) * QB],
                                    h == 0,
                                    h == NH - 1,
                                )
                            st = stDp.tile([P, QB], BF16)
                            # final block: ACT is idle (no more exps), so
                            # alternating engines halves the drain backlog
                            if qb == NQ - 1 and nb % 2 == 0:
                                nc.scalar.activation(st[:], ps[:], AF.Copy)
                            else:
                                nc.vector.tensor_copy(st[:], ps[:])
                            nc.sync.dma_start(
                                out=out[
                                    sbg * P:(sbg + 1) * P, nb * QB:(nb + 1) * QB
                                ],
                                in_=st[:],
                            )

                for qb in range(NQ):
                    nkb = (QB // P) * (qb + 1)
                    for h in range(NH):
                        # two half-bursts: keeps the o_proj drain copies from
                        # piling onto the DVE queue all at once
                        if h == 1 and qb > 0:
                            emit_D(qb - 1, sbs=(0, 1))
                        if h == 3 and qb > 0:
                            emit_D(qb - 1, sbs=(2, 3))
                        pv = pvps.tile([P, QB], F32)
                        acc = accp.tile([P, QB], BF16)
                        for kb in range(nkb):
                            t = kb - (QB // P) * qb
                            q0 = P * t if t >= 0 else 0
                            qsl = slice(q0, QB)
                            gsl = slice(qb * QB + q0, (qb + 1) * QB)
                            sc = scps.tile([P, QB], F32)
                            mm(
                                sc[:, qsl],
                                kt_a[:, kb * P:(kb + 1) * P],
                                qta_sb[:, h, gsl],
                                True,
                                False,
                            )
                            h2 = RD * (h % 2)
                            mm(
                                sc[:, qsl],
                                kt_b[h2:h2 + RD, kb * P:(kb + 1) * P],
                                qtb_sb[h2:h2 + RD, h // 2, gsl],
                                False,
                                True,
                            )
                            ex = expp.tile([P, QB], BF16)
                            nc.scalar.activation(
                                ex[:, qsl], sc[:, qsl], AF.Exp, scale=SCALE
                            )
                            if t >= 0:
                                nc.vector.tensor_mul(
                                    ex[:, q0:q0 + P], ex[:, q0:q0 + P], tri_sb[:]
                                )
                            mm(
                                pv[:, qsl],
                                v_sb[:, kb, :],
                                ex[:, qsl],
                                kb == 0,
                                kb == nkb - 1,
                            )
                            if kb == 0:
                                nc.vector.tensor_copy(acc[:], ex[:])
                            else:
                                nc.vector.tensor_add(
                                    acc[:, qsl], acc[:, qsl], ex[:, qsl]
                                )
                        bc = bcps.tile([P, QB], F32)
                        mm(bc[:], ones_sq[:], acc[:], True, True)
                        rcb = rcbp.tile([P, QB], F32)
                        nc.vector.reciprocal_approx_fast(rcb[:], bc[:])
                        nc.vector.tensor_mul(
                            out_nT[:, h, qb * QB:(qb + 1) * QB], pv[:], rcb[:]
                        )
                emit_D(NQ - 1)

    nc.compile()
    return nc


def get_nc(debug=False):
    key = bool(debug)
    if key not in _NC_CACHE:
        _NC_CACHE[key] = build_mla_nc(debug=debug)
    return _NC_CACHE[key]


def _rope_fold(W, n_heads, in_dim):
    """Fold the reference's (head-indexed) RoPE into projection columns.

    W: (in_dim, n_heads*RD). Returns W' with
    W'[:, h, 2i]   = W[:, h, 2i]*cos[h,i] - W[:, h, 2i+1]*sin[h,i]
    W'[:, h, 2i+1] = W[:, h, 2i]*sin[h,i] + W[:, h, 2i+1]*cos[h,i]
    where cos/sin use position index h (the reference bug).
    """
    freqs = 1.0 / (10000.0 ** (np.arange(0, RD, 2, dtype=np.float64) / RD))
    t = np.arange(n_heads, dtype=np.float64)
    f = np.outer(t, freqs)  # (n_heads, RD//2)
    cos = np.cos(f)
    sin = np.sin(f)
    W4 = W.astype(np.float64).reshape(in_dim, n_heads, RD // 2, 2)
    e, o = W4[..., 0], W4[..., 1]
    e2 = e * cos[None] - o * sin[None]
    o2 = e * sin[None] + o * cos[None]
    out = np.stack([e2, o2], axis=-1).reshape(in_dim, n_heads, RD)
    return out.astype(np.float32)


def _build_in_maps(hidden_states, Wq, Wkvc, Wkdec, Wvdec, Wqrope, Wkrope, Wo):
    import ml_dtypes

    bf16 = ml_dtypes.bfloat16

    Wqr = _rope_fold(np.asarray(Wqrope, np.float32), H, HID)  # (HID,H,RD)
    Wkr = _rope_fold(np.asarray(Wkrope, np.float32), KV, CD)  # (CD,KV,RD)
    Wq_full = np.concatenate(
        [np.asarray(Wq, np.float32).reshape(HID, H, HD), Wqr], axis=-1
    )  # (HID, H, DF)
    Wk_full = np.concatenate(
        [np.asarray(Wkdec, np.float32).reshape(CD, KV, HD), Wkr], axis=-1
    )  # (CD, KV, DF)
    Wv4 = np.asarray(Wvdec, np.float32).reshape(CD, KV, HD)
    Wo_eff = np.asarray(Wo, np.float32).reshape(H, DF, HID)[:, :HD, :]  # (H,HD,HID)
    Wkvc32 = np.asarray(Wkvc, np.float32)

    tri = np.tril(np.ones((P, P), np.float32)).T.astype(bf16)  # tri[p,j]=p<=j

    hs = np.asarray(hidden_states, np.float32)
    hsT_b = [np.ascontiguousarray(hs[b].T).astype(bf16) for b in range(B)]

    per_g = []
    for g in range(KV):
        Wg = Wq_full[:, NH * g:NH * (g + 1), :]  # (HID, NH, DF)
        # col order: 4 per-head nope blocks (128 each), then 2 rope-pair
        # blocks ([h0|h1] and [h2|h3], 128 each)
        qcols = np.concatenate(
            [Wg[:, h, :HD] for h in range(NH)]
            + [
                np.concatenate([Wg[:, 2 * j, HD:], Wg[:, 2 * j + 1, HD:]], axis=1)
                for j in range(NH // 2)
            ],
            axis=1,
        )  # (HID, QC)
        per_g.append(
            {
                "Wqc": np.ascontiguousarray(
                    np.concatenate([qcols, Wkvc32], axis=1)
                ).astype(bf16),
                "Wk_s": np.ascontiguousarray(Wk_full[:, g, :]).astype(bf16),
                "Wv_s": np.ascontiguousarray(Wv4[:, g, :]).astype(bf16),
                "Wo_s": np.ascontiguousarray(
                    Wo_eff[NH * g:NH * (g + 1)].reshape(NH * HD, HID)
                ).astype(bf16),
                "tri": tri,
            }
        )

    in_maps = []
    for core in range(8):
        b, g = core // 4, core % 4
        m = dict(per_g[g])
        m["hsT"] = hsT_b[b]
        in_maps.append(m)
    return in_maps


def _maybe_enable_ldw_opt():
    """Opt-in experiment: let walrus overlap LDWEIGHTS with matmuls."""
    if os.environ.get("MLA_LDWOPT") != "1":
        return
    from concourse import bass_utils as bu

    if getattr(bu, "_mla_ldw_patched", False):
        return
    orig = bu.run_command

    def patched(cmd, cwd=None):
        cmd = [
            "--enable-ldw-opt=true" if c == "--enable-ldw-opt=false" else c
            for c in cmd
        ]
        return orig(cmd, cwd=cwd)

    bu.run_command = patched
    bu._mla_ldw_patched = True


def kernel(hidden_states, Wq, Wkvc, Wkdec, Wvdec, Wqrope, Wkrope, Wo):
    from concourse.bass_utils import run_bass_kernel_spmd

    _maybe_enable_ldw_opt()

    in_maps = _build_in_maps(
        hidden_states, Wq, Wkvc, Wkdec, Wvdec, Wqrope, Wkrope, Wo
    )
    nc = get_nc(debug=False)
    trace = os.environ.get("MLA_TRACE") == "1"
    res = run_bass_kernel_spmd(nc, in_maps, list(range(8)), trace=trace)
    kernel._last_result = res
    out = np.empty((B, S, HID), np.float32)
    for b in range(B):
        acc = res.results[b * 4 + 0]["out"].astype(np.float32)
        for g in range(1, 4):
            acc = acc + res.results[b * 4 + g]["out"]
        out[b] = acc
    return out
